# revision 56
# baseline (speedup 1.0000x reference)
"""Multi-head attention (B=4, S=2048, D=512, H=8) on 8 Trainium2 NeuronCores.

Sharding: core c handles batch b = c//2 and query-half h = c%2 (1024 queries).
Each core computes q = (x_q @ Wq.T + bq)/sqrt(hd) for its queries, k/v
projections for its batch's full 2048 keys, full softmax attention for all 8
heads, and the output projection for its query rows.  Output rows across
cores are disjoint, so there are no collectives.

On-chip layout is feature-major ("transposed activations"): scores are built
directly as S^T[k, q] so the attn @ V contraction needs no transposes, and
exp(S^T) row-sums come free via a ones-column appended to V.

v2 structure (the softmax exp of 16.8M scores is the bottleneck):
  - exp is SPLIT between the Scalar engine (exact ACT exp) and the Vector
    engine (Schraudolph bit-trick exp: u16 = 184.665*s + 16251 bitcast to
    bf16, max rel err ~3.0%, which cancels mostly in the softmax normalize).
  - score matmuls are emitted qn-major so the two 64-row head-halves run
    CONCURRENTLY in disjoint PE row-groups (tile_position packing).
  - AV accumulation lags scores by one k-tile so the PE never waits on exp.
  - q/k bias drains run on ACT (Identity w/ per-partition bias AP),
    PSUM->SBUF moves for attention-out and y_acc go to DMA queues.
  - input DMAs are split across the two hardware DMA queues (sync + scalar)
    and ordered so the first matmul can start ~4us in.
"""

import numpy as np
import ml_dtypes

B = 4
S = 2048
D = 512
H = 8
HD = 64
SQ = 1024  # queries per core
N_CORES = 8

# Schraudolph exp2-bitcast constants for bf16 (7 mantissa bits):
#   code = 128*log2(e)*s + (127*128 - C + 0.5), C = 128*0.08607/2 = 5.51
EXP_A = 184.66496508959438  # 128 / ln(2)
EXP_B = 16250.99            # 16256 - 5.51 + 0.5 (truncation compensation)

_cache = {}


def _build():
    """Build (once) the SPMD Bass program shared by all 8 cores."""
    import concourse.bacc as bacc
    import concourse.mybir as mybir
    import concourse.tile as tile

    f32 = mybir.dt.float32
    bf16 = mybir.dt.bfloat16
    u16 = mybir.dt.uint16
    f8 = mybir.dt.float8e4
    PM = mybir.MatmulPerfMode
    AF = mybir.ActivationFunctionType
    OP = mybir.AluOpType

    nc = bacc.Bacc("TRN2", target_bir_lowering=False, debug=False)

    # Per-core inputs (pre-transposed / pre-cast on host).
    xqT = nc.dram_tensor("xqT", [D, SQ], f8, kind="ExternalInput").ap()
    keyT = nc.dram_tensor("keyT", [D, S], f8, kind="ExternalInput").ap()
    valT = nc.dram_tensor("valT", [D, S], bf16, kind="ExternalInput").ap()
    wqT = nc.dram_tensor("wqT", [D, D], f8, kind="ExternalInput").ap()
    wkT = nc.dram_tensor("wkT", [D, D], f8, kind="ExternalInput").ap()
    wvT = nc.dram_tensor("wvT", [D, D], bf16, kind="ExternalInput").ap()
    woT = nc.dram_tensor("woT", [D, D], bf16, kind="ExternalInput").ap()
    bqr = nc.dram_tensor("bqr", [128, 4], f32, kind="ExternalInput").ap()  # bq/8
    bkr = nc.dram_tensor("bkr", [128, 4], f32, kind="ExternalInput").ap()
    bop = nc.dram_tensor("bop", [1, D], bf16, kind="ExternalInput").ap()
    ident = nc.dram_tensor("ident", [128, 128], f32, kind="ExternalInput").ap()
    y = nc.dram_tensor("y", [SQ, D], bf16, kind="ExternalOutput").ap()

    wq_r = wqT.rearrange("(c p) e -> p c e", p=128)
    wk_r = wkT.rearrange("(c p) e -> p c e", p=128)
    wv_r = wvT.rearrange("(c p) e -> p c e", p=128)
    wo_r = woT.rearrange("(c p) e -> p c e", p=128)
    xq_r = xqT.rearrange("(c p) s -> p c s", p=128)
    key_r = keyT.rearrange("(c p) s -> p c s", p=128)
    val_r = valT.rearrange("(c p) s -> p c s", p=128)

    with tile.TileContext(nc) as tc:
        import contextlib

        with contextlib.ExitStack() as ctx:
            const = ctx.enter_context(tc.tile_pool(name="const", bufs=1))
            io = ctx.enter_context(tc.tile_pool(name="io", bufs=1))
            acts = ctx.enter_context(tc.tile_pool(name="acts", bufs=1))
            expp = ctx.enter_context(tc.tile_pool(name="expp", bufs=20))
            rpool = ctx.enter_context(tc.tile_pool(name="rpool", bufs=2))
            dramp = ctx.enter_context(
                tc.tile_pool(name="dramp", bufs=2, space="DRAM")
            )
            psA = ctx.enter_context(tc.tile_pool(name="psA", bufs=4, space="PSUM"))
            psB = ctx.enter_context(tc.tile_pool(name="psB", bufs=4, space="PSUM"))

            # ---- SBUF tensors -------------------------------------------
            # NOTE: tile-granular dependency tracking means anything read
            # early must not share a tile with late DMA/drain writes —
            # wq/wk split et0 vs rest, qT/kT one tile per et, val per slab.
            wq0_sb = const.tile([128, 4, 128], f8)
            wqR_sb = const.tile([128, 4, D - 128], f8)
            wk0_sb = const.tile([128, 4, 128], f8)
            wkR_sb = const.tile([128, 4, D - 128], f8)
            wv_sb = const.tile([128, 4, D], bf16)
            wo_sb = const.tile([128, 4, D], bf16)

            def wq_pair(p, et):
                if et == 0:
                    return wq0_sb[:, 2 * p : 2 * p + 2, :]
                return wqR_sb[:, 2 * p : 2 * p + 2,
                              (et - 1) * 128 : et * 128]

            def wk_pair(p, et):
                if et == 0:
                    return wk0_sb[:, 2 * p : 2 * p + 2, :]
                return wkR_sb[:, 2 * p : 2 * p + 2,
                              (et - 1) * 128 : et * 128]
            bq_sb = const.tile([128, 4], f32)
            bk_sb = const.tile([128, 4], f32)
            bop_sb = const.tile([1, D], bf16)
            ones_row = const.tile([1, 128], bf16)
            nc.vector.memset(ones_row[:], 1.0)
            id_sb = const.tile([128, 128], f32)
            # per-head softmax denominators, feature-major: row h = sum_k e^s
            sums_sb = const.tile([8, SQ], f32)
            nc.vector.memset(sums_sb[:], 1.0)

            xq8_sb = io.tile([128, 4, SQ], f8)
            key8_sb = io.tile([128, 4, S], f8)
            val_sl = [io.tile([128, 4, 512], bf16, name=f"val{i}")
                      for i in range(4)]

            # ---- input DMA: two hardware queues, critical-path first ----
            # sync (HWDGE, lower latency): q-path gate first, then v-path
            nc.sync.dma_start(wq0_sb[:], wq_r[:, :, 0:128])
            for dc in range(4):
                nc.sync.dma_start(xq8_sb[:, dc, :], xq_r[:, dc, :])
            nc.sync.dma_start(bq_sb[:], bqr[:])
            nc.sync.dma_start(wv_sb[:], wv_r)
            for st4 in range(4):  # val in 512-key column slabs
                sl = slice(st4 * 512, (st4 + 1) * 512)
                nc.sync.dma_start(val_sl[st4][:], val_r[:, :, sl])
            nc.sync.dma_start(wqR_sb[:], wq_r[:, :, 128:D])
            # gpsimd (SWDGE) queue: k-path, weight tails, constants
            nc.gpsimd.dma_start(wk0_sb[:], wk_r[:, :, 0:128])
            for dc in range(4):
                nc.gpsimd.dma_start(key8_sb[:, dc, :], key_r[:, dc, :])
            nc.gpsimd.dma_start(bk_sb[:], bkr[:])
            nc.gpsimd.dma_start(id_sb[:], ident[:])
            nc.gpsimd.dma_start(wkR_sb[:], wk_r[:, :, 128:D])
            nc.gpsimd.dma_start(wo_sb[:], wo_r)
            nc.gpsimd.dma_start(bop_sb[:], bop[:])

            # ---- activations (one tile per et so cross-pair projection
            # drains never falsely serialize with the current pair's
            # score matmuls) ----------------------------------------------
            qT_et = [acts.tile([128, SQ], bf16, name=f"qT{i}") for i in range(4)]
            kT_et = [acts.tile([128, S], bf16, name=f"kT{i}") for i in range(4)]
            # v natural [s, e] per k-tile, 65th column = 1.0 (row-sum trick)
            v_sb = acts.tile([128, 16, H, HD + 1], bf16)
            nc.vector.memset(v_sb[:, :, :, HD : HD + 1], 1.0)

            # q^T[e, s] = sum_d WqT[d, e] x^T[d, s]; bias+scale drain on ACT
            # (per-512 psum tiles so every psA slot is one PSUM bank).
            # Units scheduled in a pair's first kts draw scratch from psB
            # (av slots are free there) so their slow drains never block
            # the scores ring.
            def emit_qproj(et, pool=None):
                pool = pool or psA
                for qn in range(2):
                    ps = pool.tile([128, 512], f32, tag=pool.name,
                                   name=f"psq{et}_{qn}")
                    for p in range(2):
                        nc.tensor.matmul(
                            ps[:],
                            lhsT=wq_pair(p, et),
                            rhs=xq8_sb[:, 2 * p : 2 * p + 2,
                                       qn * 512 : (qn + 1) * 512],
                            start=(p == 0),
                            stop=(p == 1),
                            perf_mode=PM.DoubleRow,
                        )
                    # qT = ps*0.125/32 + bq/8  (weights host-scaled x32 to
                    # sit in e4m3's normal range; bqr pre-scaled by 1/8)
                    nc.scalar.activation(
                        qT_et[et][:, qn * 512 : (qn + 1) * 512], ps[:],
                        AF.Identity, bias=bq_sb[:, et : et + 1],
                        scale=0.125 / 32.0,
                    )

            def emit_kproj_half(et, kn, pool=None):
                pool = pool or psA
                for qn in range(2):
                    o = kn * 1024 + qn * 512
                    ps = pool.tile([128, 512], f32, tag=pool.name,
                                   name=f"psk{et}_{kn}_{qn}")
                    for p in range(2):
                        nc.tensor.matmul(
                            ps[:],
                            lhsT=wk_pair(p, et),
                            rhs=key8_sb[:, 2 * p : 2 * p + 2, o : o + 512],
                            start=(p == 0),
                            stop=(p == 1),
                            perf_mode=PM.DoubleRow,
                        )
                    nc.scalar.activation(
                        kT_et[et][:, o : o + 512], ps[:],
                        AF.Identity, bias=bk_sb[:, et : et + 1],
                        scale=1.0 / 32.0,
                    )

            # v[s, e] = sum_d v^T[d, s] WvT[d, e]   (bias folded into bo')
            def emit_vproj(st):
                psv = psA.tile([128, 512], f32, tag="psA", name=f"psv{st}")
                for dc in range(4):
                    nc.tensor.matmul(
                        psv[:],
                        lhsT=val_sl[st // 4][:, dc,
                                             (st % 4) * 128 : (st % 4 + 1) * 128],
                        rhs=wv_sb[:, dc, :],
                        start=(dc == 0),
                        stop=(dc == 3),
                    )
                # drain on ACT: vproj runs in hp0 where the PE has extra
                # matmuls per kt, so ACT has local slack there
                nc.scalar.activation(
                    v_sb[:, st, :, 0:HD],
                    psv[:].rearrange("p (h d) -> p h d", h=H),
                    AF.Copy,
                )

            # ---- scores + exp -------------------------------------------
            # Scores are emitted qn-major so the hh=0 (rows 0-63) and hh=1
            # (rows 64-127) matmuls land in disjoint PE row groups.  The
            # exp of each [128,1024] score tile is split into two [128,512]
            # query-half ops running CONCURRENTLY on ACT (exact exp) and
            # DVE (Schraudolph u16 bitcast) — the first halves start right
            # after the first score wave, which releases the PSUM score
            # slot ~0.4us earlier and shortens the scores->exp->scores
            # pipeline ring that otherwise throttles the PE.
            def emit_exp_qc(hp, kt, st_q, exp_tiles, qc):
                # (hh0,qc0)+(hh1,qc1) -> ACT; (hh1,qc0)+(hh0,qc1) -> DVE
                act_hh, dve_hh = (0, 1) if qc == 0 else (1, 0)
                ea = expp.tile([128, 512], bf16, tag="exp",
                               name=f"e{hp}_{kt}_{act_hh}_{qc}")
                nc.scalar.activation(ea[:], st_q[act_hh][:], AF.Exp)
                exp_tiles[act_hh][qc][kt] = (ea, False)
                ed = expp.tile([128, 512], u16, tag="exp",
                               name=f"e{hp}_{kt}_{dve_hh}_{qc}")
                nc.vector.tensor_scalar(
                    ed[:], st_q[dve_hh][:], EXP_A, EXP_B, OP.mult, OP.add,
                )
                exp_tiles[dve_hh][qc][kt] = (ed, True)

            def emit_scores_exp(hp, kt, exp_tiles):
                # one PSUM tile per (hh, qn) quarter: distinct tiles keep
                # the qn1 matmuls free of false dependencies against the
                # qn0 exps, and give the scores ring 4 fine-grained slots
                for qn in range(2):
                    st_q = [
                        psA.tile([128, 512], f32, tag="psA",
                                 name=f"st{hp}_{kt}_{hh}_{qn}")
                        for hh in range(2)
                    ]
                    for hh in range(2):
                        lo = 64 * hh
                        nc.tensor.matmul(
                            st_q[hh][:],
                            lhsT=kT_et[hp][lo : lo + 64,
                                           kt * 128 : (kt + 1) * 128],
                            rhs=qT_et[hp][lo : lo + 64,
                                          qn * 512 : (qn + 1) * 512],
                            start=True,
                            stop=True,
                            tile_position=(lo, 0),
                        )
                    emit_exp_qc(hp, kt, st_q, exp_tiles, qn)

            def av_rhs(exp_tiles, hh, kt, qc):
                e, is_u16 = exp_tiles[hh][qc][kt]
                return e[:].bitcast(bf16) if is_u16 else e[:]

            # ---- per-pair normalization ---------------------------------
            pending_norm = {}

            def emit_recip(hp):
                # invert softmax denominators in a TRANSPOSED layout: a
                # [1,1024] DVE reciprocal is ~6.5us (one lane); transposing
                # via the PE makes it [128,64] (~0.5us).  Scratch comes from
                # the psA ring (1-bank slots, short-lived) so the psB ring
                # stays av-only and can never deadlock.
                tp = psA.tile([128, 8, 8], f32, tag="psA", name=f"tp{hp}")
                for b in range(8):
                    nc.tensor.transpose(
                        tp[:, b, :],
                        sums_sb[0:8, b * 128 : (b + 1) * 128],
                        id_sb[0:8, 0:8],
                    )
                rcp = rpool.tile([128, 8, 8], f32, tag="rcp", name=f"rcp{hp}")
                nc.vector.reciprocal(rcp[:], tp[:])
                return rcp

            def emit_norm(hp):
                pair_out, avsbs = pending_norm.pop(hp)
                rcp = emit_recip(hp)
                for hh in range(2):
                    rback = psA.tile([8, 128], f32, tag="psA",
                                     name=f"rback{hp}_{hh}")
                    nc.tensor.transpose(
                        rback[:], rcp[:, :, 2 * hp + hh], id_sb[0:128, :]
                    )
                    rr8 = rpool.tile([8, 128], f32, tag="rr8",
                                     name=f"rr8{hp}_{hh}")
                    nc.vector.tensor_copy(rr8[:], rback[:])
                    scr = dramp.tile([1, SQ], f32, tag="scr",
                                     name=f"scr{hp}_{hh}")
                    nc.sync.dma_start(
                        scr[:].rearrange("x (a b) -> (x a) b", a=8), rr8[:]
                    )
                    rb = rpool.tile([64, SQ], f32, tag="rb",
                                    name=f"rb{hp}_{hh}")
                    nc.sync.dma_start(rb[:], scr[:].to_broadcast((64, SQ)))
                    nc.vector.tensor_tensor(
                        pair_out[64 * hh : 64 * hh + 64, :],
                        avsbs[hh][0:HD, :],
                        rb[:],
                        OP.mult,
                    )

            # ---- main attention loop ------------------------------------
            emit_qproj(0)
            emit_kproj_half(0, 0)
            emit_kproj_half(0, 1)
            # Projection units for the next pair run in the FIRST kts of
            # pairs 1-2 (where the PE has no AV work yet thanks to the AV
            # lag) so pair boundaries never leave the PE idle long enough
            # to trip the HAM clock throttle.  hp0 keeps them late (it is
            # already PE-rich with the vproj stream).
            proj_sched = {
                (0, 5): lambda: emit_qproj(1),
                (0, 9): lambda: emit_kproj_half(1, 0),
                (0, 13): lambda: emit_kproj_half(1, 1),
                (1, 0): lambda: emit_qproj(2, psB),
                (1, 1): lambda: emit_kproj_half(2, 0, psB),
                (1, 2): lambda: emit_kproj_half(2, 1, psB),
                (2, 0): lambda: emit_qproj(3, psB),
                (2, 1): lambda: emit_kproj_half(3, 0, psB),
                (2, 2): lambda: emit_kproj_half(3, 1, psB),
            }

            # out-projection stage A: pairs 0-2 + bias -> y_acc (SBUF)
            y_acc = acts.tile([128, 8, 512], f32)

            def emit_outA(stq):
                psy = psA.tile([128, 512], f32, tag="psA", name=f"psyA{stq}")
                for c in range(3):
                    nc.tensor.matmul(
                        psy[:],
                        lhsT=outT[c][:, stq * 128 : (stq + 1) * 128],
                        rhs=wo_sb[:, c, :],
                        start=(c == 0),
                        stop=False,
                    )
                nc.tensor.matmul(
                    psy[:], lhsT=ones_row[:], rhs=bop_sb[:], start=False,
                    stop=True,
                )
                # alternate drain engine so neither ACT nor DVE builds a
                # backlog during pair 3 (a late exp backlog re-throttles
                # the PE right before the tail)
                if stq % 2 == 0:
                    nc.scalar.activation(y_acc[:, stq, :], psy[:], AF.Copy)
                else:
                    nc.vector.tensor_copy(y_acc[:, stq, :], psy[:])

            outT = []  # 4 pair tiles [128, SQ] = attn-out^T (normalized)
            for hp in range(4):
                pair_out = acts.tile([128, SQ], bf16, tag=f"outT{hp}")
                outT.append(pair_out)
                av = [[None, None], [None, None]]
                exp_tiles = [[[None] * 16, [None] * 16],
                             [[None] * 16, [None] * 16]]

                def emit_av(kt):
                    for hh in range(2):
                        h = 2 * hp + hh
                        for qc in range(2):
                            if av[hh][qc] is None:
                                # allocated lazily AFTER the previous pair's
                                # norm grabbed its psB scratch slots
                                av[hh][qc] = psB.tile(
                                    [HD + 1, 512], f32, tag="psB",
                                    name=f"av{hp}_{hh}_{qc}",
                                )
                            nc.tensor.matmul(
                                av[hh][qc][:],
                                lhsT=v_sb[:, kt, h, :],
                                rhs=av_rhs(exp_tiles, hh, kt, qc),
                                start=(kt == 0),
                                stop=(kt == 15),
                            )

                for kt in range(16):
                    emit_scores_exp(hp, kt, exp_tiles)
                    if hp == 0:
                        emit_vproj(kt)
                    if (hp, kt) in proj_sched:
                        proj_sched[(hp, kt)]()
                    if kt == 3 and (hp - 1) in pending_norm:
                        # norm of the previous pair (sums DMA has landed by
                        # now; scratch from the psA ring)
                        emit_norm(hp - 1)
                    if hp == 3 and 10 <= kt <= 15:
                        # out-projection stage A (pairs 0-2 + bias) rides in
                        # the BACK half of pair 3's kt loop: keeps the PE
                        # dense right up to the tail so the HAM clock never
                        # throttles before the stage-B matmuls
                        emit_outA(kt - 10)
                    # AV lags scores by two k-tiles so the PE never waits on
                    # the exp engines.
                    if kt > 1:
                        emit_av(kt - 2)
                emit_av(14)
                emit_av(15)
                if hp == 3:
                    # last two stage-A units bridge the PE-idle window while
                    # the trailing exps and the pair-3 reciprocal chain
                    # drain, so the clock stays warm into stage B
                    emit_outA(6)
                    emit_outA(7)

                if hp == 3:
                    # stage B consumes pair 3 unnormalized (scaled per head
                    # by the transposed reciprocal after its projection).
                    # Denominators: engine writes must start at an aligned
                    # partition, so stage the four row-HD pieces on
                    # partition 0, then one DMA scatter into sums rows 6:8.
                    # s3 (denominators) on DVE so the reciprocal chain is
                    # not queued behind ACT's trailing exps; outT copies on
                    # ACT (needed later, by the stage-B matmuls)
                    # s3 on ACT (its exp queue drains first, so the
                    # reciprocal chain starts immediately); outT copies on
                    # DVE in parallel
                    s3 = rpool.tile([1, 2, 2, 512], f32, tag="s3", name="s3")
                    for hh in range(2):
                        for qc in range(2):
                            nc.scalar.activation(
                                s3[:, hh, qc, :],
                                av[hh][qc][HD : HD + 1, :],
                                AF.Copy,
                            )
                    for hh in range(2):
                        for qc in range(2):
                            nc.vector.tensor_copy(
                                pair_out[64 * hh : 64 * hh + HD,
                                         qc * 512 : (qc + 1) * 512],
                                av[hh][qc][0:HD, :],
                            )
                    # NOTE: partition-expanding rearrange DMAs are broken on
                    # HW (sim-only); use one shape-preserving DMA per row.
                    for hh in range(2):
                        nc.gpsimd.dma_start(
                            sums_sb[6 + hh : 7 + hh, :],
                            s3[:, hh, :, :].rearrange("p q s -> p (q s)"),
                        )
                else:
                    # drain PSUM accumulators (65 rows incl. the denominator
                    # row); hh=0 on ACT, hh=1 on DVE; denominator row to
                    # sums_sb via the gpsimd DMA queue (SBUF->SBUF).
                    avsbs = []
                    for hh in range(2):
                        avsb = rpool.tile([HD + 1, SQ], f32, tag="avsb",
                                          name=f"avsb{hp}_{hh}")
                        avsbs.append(avsb)
                        for qc in range(2):
                            if hh == 0:
                                nc.scalar.activation(
                                    avsb[:, qc * 512 : (qc + 1) * 512],
                                    av[hh][qc][:], AF.Copy,
                                )
                            else:
                                nc.vector.tensor_copy(
                                    avsb[:, qc * 512 : (qc + 1) * 512],
                                    av[hh][qc][:],
                                )
                        nc.gpsimd.dma_start(
                            sums_sb[2 * hp + hh : 2 * hp + hh + 1, :],
                            avsb[HD : HD + 1, :],
                        )
                    pending_norm[hp] = (pair_out, avsbs)

            # ---- output projection stage B ------------------------------
            # y[q, o] = y_acc[q, o] + sum_{pair3} outT3[e, q] WoT[e, o]/d3
            rcp3 = emit_recip(3)
            for stq in range(8):
                ysb = rpool.tile([128, 512], bf16, tag="ysb", name=f"ysb{stq}")
                for hh in range(2):
                    psy = psB.tile([128, 512], f32, tag="psB",
                                   name=f"psyB{stq}_{hh}")
                    nc.tensor.matmul(
                        psy[:],
                        lhsT=outT[3][64 * hh : 64 * hh + 64,
                                     stq * 128 : (stq + 1) * 128],
                        rhs=wo_sb[64 * hh : 64 * hh + 64, 3, :],
                        start=True,
                        stop=True,
                    )
                    nc.vector.scalar_tensor_tensor(
                        ysb[:],
                        psy[:],
                        rcp3[:, stq, 6 + hh : 7 + hh],
                        y_acc[:, stq, :] if hh == 0 else ysb[:],
                        OP.mult,
                        OP.add,
                    )
                nc.sync.dma_start(y[stq * 128 : (stq + 1) * 128, :], ysb[:])

    nc.compile()
    return nc


def _get_nc():
    if "nc" not in _cache:
        _cache["nc"] = _build()
    return _cache["nc"]


def _host_prep(query, key, value, Wq, bq, Wk, bk, Wv, bv, Wo, bo):
    """Shard + transpose + cast inputs for the 8 cores."""
    bf = ml_dtypes.bfloat16
    f8 = ml_dtypes.float8_e4m3
    wqT = np.ascontiguousarray(Wq.T * 32.0).astype(f8)
    wkT = np.ascontiguousarray(Wk.T * 32.0).astype(f8)
    wvT = np.ascontiguousarray(Wv.T).astype(bf)
    woT = np.ascontiguousarray(Wo.T).astype(bf)
    bqr = np.ascontiguousarray((bq * 0.125).reshape(4, 128).T).astype(np.float32)
    bkr = np.ascontiguousarray(bk.reshape(4, 128).T).astype(np.float32)
    bop = (bo + Wo @ bv).astype(np.float32).reshape(1, D).astype(bf)
    ident = np.eye(128, dtype=np.float32)

    in_maps = []
    for c in range(N_CORES):
        b, half = divmod(c, 2)
        xqT = np.ascontiguousarray(
            query[b, half * SQ : (half + 1) * SQ, :].T
        ).astype(f8)
        keyT = np.ascontiguousarray(key[b].T).astype(f8)
        valT = np.ascontiguousarray(value[b].T).astype(bf)
        in_maps.append(
            {
                "xqT": xqT, "keyT": keyT, "valT": valT,
                "wqT": wqT, "wkT": wkT, "wvT": wvT, "woT": woT,
                "bqr": bqr, "bkr": bkr, "bop": bop, "ident": ident,
            }
        )
    return in_maps


def _assemble(results):
    out = np.empty((B, S, D), np.float32)
    for c in range(N_CORES):
        b, half = divmod(c, 2)
        out[b, half * SQ : (half + 1) * SQ, :] = results[c]["y"]
    return out


def _run(in_maps, **spmd_kwargs):
    from concourse.bass_utils import run_bass_kernel_spmd

    nc = _get_nc()
    return run_bass_kernel_spmd(nc, in_maps, list(range(N_CORES)), **spmd_kwargs)


def _reference_fallback(query, key, value, mask, Wq, bq, Wk, bk, Wv, bv, Wo, bo):
    """Exact numpy path, used only if the mask is not all-ones."""
    q = (query @ Wq.T + bq).reshape(B, S, H, HD).transpose(0, 2, 1, 3)
    k = (key @ Wk.T + bk).reshape(B, S, H, HD).transpose(0, 2, 1, 3)
    v = (value @ Wv.T + bv).reshape(B, S, H, HD).transpose(0, 2, 1, 3)
    scores = np.einsum("bhqd,bhkd->bhqk", q, k) / np.sqrt(HD).astype(np.float32)
    scores = np.where(mask[:, None, :, :] == 0, -np.inf, scores)
    scores = scores - scores.max(axis=-1, keepdims=True)
    e = np.exp(scores)
    attn = e / e.sum(axis=-1, keepdims=True)
    x = np.einsum("bhqk,bhkd->bhqd", attn, v)
    x = x.transpose(0, 2, 1, 3).reshape(B, S, D)
    return (x @ Wo.T + bo).astype(np.float32)


def kernel(query, key, value, mask, Wq, bq, Wk, bk, Wv, bv, Wo, bo):
    query = np.asarray(query, np.float32)
    key = np.asarray(key, np.float32)
    value = np.asarray(value, np.float32)
    mask_np = np.asarray(mask)
    args = [
        np.asarray(a, np.float32)
        for a in (Wq, bq, Wk, bk, Wv, bv, Wo, bo)
    ]
    if not np.all(mask_np != 0):
        return _reference_fallback(query, key, value, mask_np, *args)
    in_maps = _host_prep(query, key, value, *args)
    res = _run(in_maps, trace=False)
    return _assemble(res.results)


# revision 57
# speedup vs baseline: 1.0042x; 1.0042x over previous
"""Multi-head attention (B=4, S=2048, D=512, H=8) on 8 Trainium2 NeuronCores.

Sharding: core c handles batch b = c//2 and query-half h = c%2 (1024 queries).
Each core computes q = (x_q @ Wq.T + bq)/sqrt(hd) for its queries, k/v
projections for its batch's full 2048 keys, full softmax attention for all 8
heads, and the output projection for its query rows.  Output rows across
cores are disjoint, so there are no collectives.

On-chip layout is feature-major ("transposed activations"): scores are built
directly as S^T[k, q] so the attn @ V contraction needs no transposes, and
exp(S^T) row-sums come free via a ones-column appended to V.

v2 structure (the softmax exp of 16.8M scores is the bottleneck):
  - exp is SPLIT between the Scalar engine (exact ACT exp) and the Vector
    engine (Schraudolph bit-trick exp: u16 = 184.665*s + 16251 bitcast to
    bf16, max rel err ~3.0%, which cancels mostly in the softmax normalize).
  - score matmuls are emitted qn-major so the two 64-row head-halves run
    CONCURRENTLY in disjoint PE row-groups (tile_position packing).
  - AV accumulation lags scores by one k-tile so the PE never waits on exp.
  - q/k bias drains run on ACT (Identity w/ per-partition bias AP),
    PSUM->SBUF moves for attention-out and y_acc go to DMA queues.
  - input DMAs are split across the two hardware DMA queues (sync + scalar)
    and ordered so the first matmul can start ~4us in.
"""

import numpy as np
import ml_dtypes

B = 4
S = 2048
D = 512
H = 8
HD = 64
SQ = 1024  # queries per core
N_CORES = 8

# Schraudolph exp2-bitcast constants for bf16 (7 mantissa bits):
#   code = 128*log2(e)*s + (127*128 - C + 0.5), C = 128*0.08607/2 = 5.51
EXP_A = 184.66496508959438  # 128 / ln(2)
EXP_B = 16250.99            # 16256 - 5.51 + 0.5 (truncation compensation)

_cache = {}


def _build():
    """Build (once) the SPMD Bass program shared by all 8 cores."""
    import concourse.bacc as bacc
    import concourse.mybir as mybir
    import concourse.tile as tile

    f32 = mybir.dt.float32
    bf16 = mybir.dt.bfloat16
    u16 = mybir.dt.uint16
    f8 = mybir.dt.float8e4
    PM = mybir.MatmulPerfMode
    AF = mybir.ActivationFunctionType
    OP = mybir.AluOpType

    nc = bacc.Bacc("TRN2", target_bir_lowering=False, debug=False)

    # Per-core inputs (pre-transposed / pre-cast on host).
    xqT = nc.dram_tensor("xqT", [D, SQ], f8, kind="ExternalInput").ap()
    keyT = nc.dram_tensor("keyT", [D, S], f8, kind="ExternalInput").ap()
    valT = nc.dram_tensor("valT", [D, S], bf16, kind="ExternalInput").ap()
    wqT = nc.dram_tensor("wqT", [D, D], f8, kind="ExternalInput").ap()
    wkT = nc.dram_tensor("wkT", [D, D], f8, kind="ExternalInput").ap()
    wvT = nc.dram_tensor("wvT", [D, D], bf16, kind="ExternalInput").ap()
    woT = nc.dram_tensor("woT", [D, D], bf16, kind="ExternalInput").ap()
    bqr = nc.dram_tensor("bqr", [128, 4], f32, kind="ExternalInput").ap()  # bq/8
    bkr = nc.dram_tensor("bkr", [128, 4], f32, kind="ExternalInput").ap()
    bop = nc.dram_tensor("bop", [1, D], bf16, kind="ExternalInput").ap()
    ident = nc.dram_tensor("ident", [128, 128], f32, kind="ExternalInput").ap()
    y = nc.dram_tensor("y", [SQ, D], bf16, kind="ExternalOutput").ap()

    wq_r = wqT.rearrange("(c p) e -> p c e", p=128)
    wk_r = wkT.rearrange("(c p) e -> p c e", p=128)
    wv_r = wvT.rearrange("(c p) e -> p c e", p=128)
    wo_r = woT.rearrange("(c p) e -> p c e", p=128)
    xq_r = xqT.rearrange("(c p) s -> p c s", p=128)
    key_r = keyT.rearrange("(c p) s -> p c s", p=128)
    val_r = valT.rearrange("(c p) s -> p c s", p=128)

    with tile.TileContext(nc) as tc:
        import contextlib

        with contextlib.ExitStack() as ctx:
            const = ctx.enter_context(tc.tile_pool(name="const", bufs=1))
            io = ctx.enter_context(tc.tile_pool(name="io", bufs=1))
            acts = ctx.enter_context(tc.tile_pool(name="acts", bufs=1))
            expp = ctx.enter_context(tc.tile_pool(name="expp", bufs=20))
            rpool = ctx.enter_context(tc.tile_pool(name="rpool", bufs=2))
            dramp = ctx.enter_context(
                tc.tile_pool(name="dramp", bufs=2, space="DRAM")
            )
            psA = ctx.enter_context(tc.tile_pool(name="psA", bufs=4, space="PSUM"))
            psB = ctx.enter_context(tc.tile_pool(name="psB", bufs=4, space="PSUM"))

            # ---- SBUF tensors -------------------------------------------
            # NOTE: tile-granular dependency tracking means anything read
            # early must not share a tile with late DMA/drain writes —
            # wq/wk split et0 vs rest, qT/kT one tile per et, val per slab.
            wq0_sb = const.tile([128, 4, 128], f8)
            wqR_sb = const.tile([128, 4, D - 128], f8)
            wk0_sb = const.tile([128, 4, 128], f8)
            wkR_sb = const.tile([128, 4, D - 128], f8)
            wv_sb = const.tile([128, 4, D], bf16)
            wo_sb = const.tile([128, 4, D], bf16)

            def wq_pair(p, et):
                if et == 0:
                    return wq0_sb[:, 2 * p : 2 * p + 2, :]
                return wqR_sb[:, 2 * p : 2 * p + 2,
                              (et - 1) * 128 : et * 128]

            def wk_pair(p, et):
                if et == 0:
                    return wk0_sb[:, 2 * p : 2 * p + 2, :]
                return wkR_sb[:, 2 * p : 2 * p + 2,
                              (et - 1) * 128 : et * 128]
            bq_sb = const.tile([128, 4], f32)
            bk_sb = const.tile([128, 4], f32)
            bop_sb = const.tile([1, D], bf16)
            ones_row = const.tile([1, 128], bf16)
            nc.vector.memset(ones_row[:], 1.0)
            id_sb = const.tile([128, 128], f32)
            # per-head softmax denominators, feature-major: row h = sum_k e^s
            sums_sb = const.tile([8, SQ], f32)
            nc.vector.memset(sums_sb[:], 1.0)

            xq8_sb = io.tile([128, 4, SQ], f8)
            key8_sb = io.tile([128, 4, S], f8)
            val_sl = [io.tile([128, 4, 512], bf16, name=f"val{i}")
                      for i in range(4)]

            # ---- input DMA: two hardware queues, critical-path first ----
            # sync (HWDGE, lower latency): q-path gate first, then v-path
            nc.sync.dma_start(wq0_sb[:], wq_r[:, :, 0:128])
            for dc in range(4):
                nc.sync.dma_start(xq8_sb[:, dc, :], xq_r[:, dc, :])
            nc.sync.dma_start(bq_sb[:], bqr[:])
            nc.sync.dma_start(wv_sb[:], wv_r)
            for st4 in range(4):  # val in 512-key column slabs
                sl = slice(st4 * 512, (st4 + 1) * 512)
                nc.sync.dma_start(val_sl[st4][:], val_r[:, :, sl])
            nc.sync.dma_start(wqR_sb[:], wq_r[:, :, 128:D])
            # gpsimd (SWDGE) queue: k-path, weight tails, constants
            nc.gpsimd.dma_start(wk0_sb[:], wk_r[:, :, 0:128])
            for dc in range(4):
                nc.gpsimd.dma_start(key8_sb[:, dc, :], key_r[:, dc, :])
            nc.gpsimd.dma_start(bk_sb[:], bkr[:])
            nc.gpsimd.dma_start(id_sb[:], ident[:])
            nc.gpsimd.dma_start(wkR_sb[:], wk_r[:, :, 128:D])
            nc.gpsimd.dma_start(wo_sb[:], wo_r)
            nc.gpsimd.dma_start(bop_sb[:], bop[:])

            # ---- activations (one tile per et so cross-pair projection
            # drains never falsely serialize with the current pair's
            # score matmuls) ----------------------------------------------
            qT_et = [acts.tile([128, SQ], bf16, name=f"qT{i}") for i in range(4)]
            kT_et = [acts.tile([128, S], bf16, name=f"kT{i}") for i in range(4)]
            # v natural [s, e] per k-tile, 65th column = 1.0 (row-sum trick)
            v_sb = acts.tile([128, 16, H, HD + 1], bf16)
            nc.vector.memset(v_sb[:, :, :, HD : HD + 1], 1.0)

            # q^T[e, s] = sum_d WqT[d, e] x^T[d, s]; bias+scale drain on ACT
            # (per-512 psum tiles so every psA slot is one PSUM bank).
            # Units scheduled in a pair's first kts draw scratch from psB
            # (av slots are free there) so their slow drains never block
            # the scores ring.
            def emit_qproj(et, pool=None):
                pool = pool or psA
                for qn in range(2):
                    ps = pool.tile([128, 512], f32, tag=pool.name,
                                   name=f"psq{et}_{qn}")
                    for p in range(2):
                        nc.tensor.matmul(
                            ps[:],
                            lhsT=wq_pair(p, et),
                            rhs=xq8_sb[:, 2 * p : 2 * p + 2,
                                       qn * 512 : (qn + 1) * 512],
                            start=(p == 0),
                            stop=(p == 1),
                            perf_mode=PM.DoubleRow,
                        )
                    # qT = ps*0.125/32 + bq/8  (weights host-scaled x32 to
                    # sit in e4m3's normal range; bqr pre-scaled by 1/8)
                    nc.scalar.activation(
                        qT_et[et][:, qn * 512 : (qn + 1) * 512], ps[:],
                        AF.Identity, bias=bq_sb[:, et : et + 1],
                        scale=0.125 / 32.0,
                    )

            def emit_kproj_half(et, kn, pool=None):
                pool = pool or psA
                for qn in range(2):
                    o = kn * 1024 + qn * 512
                    ps = pool.tile([128, 512], f32, tag=pool.name,
                                   name=f"psk{et}_{kn}_{qn}")
                    for p in range(2):
                        nc.tensor.matmul(
                            ps[:],
                            lhsT=wk_pair(p, et),
                            rhs=key8_sb[:, 2 * p : 2 * p + 2, o : o + 512],
                            start=(p == 0),
                            stop=(p == 1),
                            perf_mode=PM.DoubleRow,
                        )
                    nc.scalar.activation(
                        kT_et[et][:, o : o + 512], ps[:],
                        AF.Identity, bias=bk_sb[:, et : et + 1],
                        scale=1.0 / 32.0,
                    )

            # v[s, e] = sum_d v^T[d, s] WvT[d, e]   (bias folded into bo')
            def emit_vproj(st):
                psv = psA.tile([128, 512], f32, tag="psA", name=f"psv{st}")
                for dc in range(4):
                    nc.tensor.matmul(
                        psv[:],
                        lhsT=val_sl[st // 4][:, dc,
                                             (st % 4) * 128 : (st % 4 + 1) * 128],
                        rhs=wv_sb[:, dc, :],
                        start=(dc == 0),
                        stop=(dc == 3),
                    )
                # drain on ACT: vproj runs in hp0 where the PE has extra
                # matmuls per kt, so ACT has local slack there
                nc.scalar.activation(
                    v_sb[:, st, :, 0:HD],
                    psv[:].rearrange("p (h d) -> p h d", h=H),
                    AF.Copy,
                )

            # ---- scores + exp -------------------------------------------
            # Scores are emitted qn-major so the hh=0 (rows 0-63) and hh=1
            # (rows 64-127) matmuls land in disjoint PE row groups.  The
            # exp of each [128,1024] score tile is split into two [128,512]
            # query-half ops running CONCURRENTLY on ACT (exact exp) and
            # DVE (Schraudolph u16 bitcast) — the first halves start right
            # after the first score wave, which releases the PSUM score
            # slot ~0.4us earlier and shortens the scores->exp->scores
            # pipeline ring that otherwise throttles the PE.
            def emit_exp_qc(hp, kt, st_q, exp_tiles, qc):
                # (hh0,qc0)+(hh1,qc1) -> ACT; (hh1,qc0)+(hh0,qc1) -> DVE
                act_hh, dve_hh = (0, 1) if qc == 0 else (1, 0)
                ea = expp.tile([128, 512], bf16, tag="exp",
                               name=f"e{hp}_{kt}_{act_hh}_{qc}")
                nc.scalar.activation(ea[:], st_q[act_hh][:], AF.Exp)
                exp_tiles[act_hh][qc][kt] = (ea, False)
                ed = expp.tile([128, 512], u16, tag="exp",
                               name=f"e{hp}_{kt}_{dve_hh}_{qc}")
                nc.vector.tensor_scalar(
                    ed[:], st_q[dve_hh][:], EXP_A, EXP_B, OP.mult, OP.add,
                )
                exp_tiles[dve_hh][qc][kt] = (ed, True)

            def emit_scores_exp(hp, kt, exp_tiles):
                # one PSUM tile per (hh, qn) quarter: distinct tiles keep
                # the qn1 matmuls free of false dependencies against the
                # qn0 exps, and give the scores ring 4 fine-grained slots
                for qn in range(2):
                    st_q = [
                        psA.tile([128, 512], f32, tag="psA",
                                 name=f"st{hp}_{kt}_{hh}_{qn}")
                        for hh in range(2)
                    ]
                    for hh in range(2):
                        lo = 64 * hh
                        nc.tensor.matmul(
                            st_q[hh][:],
                            lhsT=kT_et[hp][lo : lo + 64,
                                           kt * 128 : (kt + 1) * 128],
                            rhs=qT_et[hp][lo : lo + 64,
                                          qn * 512 : (qn + 1) * 512],
                            start=True,
                            stop=True,
                            tile_position=(lo, 0),
                        )
                    emit_exp_qc(hp, kt, st_q, exp_tiles, qn)

            def av_rhs(exp_tiles, hh, kt, qc):
                e, is_u16 = exp_tiles[hh][qc][kt]
                return e[:].bitcast(bf16) if is_u16 else e[:]

            # ---- per-pair normalization ---------------------------------
            pending_norm = {}

            def emit_recip(hp):
                # invert softmax denominators in a TRANSPOSED layout: a
                # [1,1024] DVE reciprocal is ~6.5us (one lane); transposing
                # via the PE makes it [128,64] (~0.5us).  Scratch comes from
                # the psA ring (1-bank slots, short-lived) so the psB ring
                # stays av-only and can never deadlock.
                tp = psA.tile([128, 8, 8], f32, tag="psA", name=f"tp{hp}")
                for b in range(8):
                    nc.tensor.transpose(
                        tp[:, b, :],
                        sums_sb[0:8, b * 128 : (b + 1) * 128],
                        id_sb[0:8, 0:8],
                    )
                rcp = rpool.tile([128, 8, 8], f32, tag="rcp", name=f"rcp{hp}")
                nc.vector.reciprocal(rcp[:], tp[:])
                return rcp

            def emit_norm(hp):
                pair_out, avsbs = pending_norm.pop(hp)
                rcp = emit_recip(hp)
                for hh in range(2):
                    rback = psA.tile([8, 128], f32, tag="psA",
                                     name=f"rback{hp}_{hh}")
                    nc.tensor.transpose(
                        rback[:], rcp[:, :, 2 * hp + hh], id_sb[0:128, :]
                    )
                    rr8 = rpool.tile([8, 128], f32, tag="rr8",
                                     name=f"rr8{hp}_{hh}")
                    nc.vector.tensor_copy(rr8[:], rback[:])
                    scr = dramp.tile([1, SQ], f32, tag="scr",
                                     name=f"scr{hp}_{hh}")
                    nc.sync.dma_start(
                        scr[:].rearrange("x (a b) -> (x a) b", a=8), rr8[:]
                    )
                    rb = rpool.tile([64, SQ], f32, tag="rb",
                                    name=f"rb{hp}_{hh}")
                    nc.sync.dma_start(rb[:], scr[:].to_broadcast((64, SQ)))
                    nc.vector.tensor_tensor(
                        pair_out[64 * hh : 64 * hh + 64, :],
                        avsbs[hh][0:HD, :],
                        rb[:],
                        OP.mult,
                    )

            # ---- main attention loop ------------------------------------
            emit_qproj(0)
            emit_kproj_half(0, 0)
            emit_kproj_half(0, 1)
            # Projection units for the next pair run in the FIRST kts of
            # pairs 1-2 (where the PE has no AV work yet thanks to the AV
            # lag) so pair boundaries never leave the PE idle long enough
            # to trip the HAM clock throttle.  hp0 keeps them late (it is
            # already PE-rich with the vproj stream).
            proj_sched = {
                (0, 5): lambda: emit_qproj(1),
                (0, 9): lambda: emit_kproj_half(1, 0),
                (0, 13): lambda: emit_kproj_half(1, 1),
                (1, 0): lambda: emit_qproj(2, psB),
                (1, 1): lambda: emit_kproj_half(2, 0, psB),
                (1, 2): lambda: emit_kproj_half(2, 1, psB),
                (2, 0): lambda: emit_qproj(3, psB),
                (2, 1): lambda: emit_kproj_half(3, 0, psB),
                (2, 2): lambda: emit_kproj_half(3, 1, psB),
            }

            # out-projection stage A: pairs 0-2 + bias -> y_acc (SBUF)
            y_acc = acts.tile([128, 8, 512], f32)

            def emit_outA(stq):
                psy = psA.tile([128, 512], f32, tag="psA", name=f"psyA{stq}")
                for c in range(3):
                    nc.tensor.matmul(
                        psy[:],
                        lhsT=outT[c][:, stq * 128 : (stq + 1) * 128],
                        rhs=wo_sb[:, c, :],
                        start=(c == 0),
                        stop=False,
                    )
                nc.tensor.matmul(
                    psy[:], lhsT=ones_row[:], rhs=bop_sb[:], start=False,
                    stop=True,
                )
                # alternate drain engine so neither ACT nor DVE builds a
                # backlog during pair 3 (a late exp backlog re-throttles
                # the PE right before the tail)
                if stq % 2 == 0:
                    nc.scalar.activation(y_acc[:, stq, :], psy[:], AF.Copy)
                else:
                    nc.vector.tensor_copy(y_acc[:, stq, :], psy[:])

            outT = []  # 4 pair tiles [128, SQ] = attn-out^T (normalized)
            for hp in range(4):
                pair_out = acts.tile([128, SQ], bf16, tag=f"outT{hp}")
                outT.append(pair_out)
                av = [[None, None], [None, None]]
                exp_tiles = [[[None] * 16, [None] * 16],
                             [[None] * 16, [None] * 16]]

                def emit_av(kt):
                    for hh in range(2):
                        h = 2 * hp + hh
                        for qc in range(2):
                            if av[hh][qc] is None:
                                # allocated lazily AFTER the previous pair's
                                # norm grabbed its psB scratch slots
                                av[hh][qc] = psB.tile(
                                    [HD + 1, 512], f32, tag="psB",
                                    name=f"av{hp}_{hh}_{qc}",
                                )
                            nc.tensor.matmul(
                                av[hh][qc][:],
                                lhsT=v_sb[:, kt, h, :],
                                rhs=av_rhs(exp_tiles, hh, kt, qc),
                                start=(kt == 0),
                                stop=(kt == 15),
                            )

                for kt in range(16):
                    if hp == 0:
                        # vproj BEFORE scores: its ACT drain then precedes
                        # this kt's exps in the queue, releasing the psv
                        # PSUM slot ~0.7us earlier (the slot's next user
                        # stalled on it every hp0 kt)
                        emit_vproj(kt)
                    emit_scores_exp(hp, kt, exp_tiles)
                    if (hp, kt) in proj_sched:
                        proj_sched[(hp, kt)]()
                    if kt == 3 and (hp - 1) in pending_norm:
                        # norm of the previous pair (sums DMA has landed by
                        # now; scratch from the psA ring)
                        emit_norm(hp - 1)
                    if hp == 3 and 10 <= kt <= 15:
                        # out-projection stage A (pairs 0-2 + bias) rides in
                        # the BACK half of pair 3's kt loop: keeps the PE
                        # dense right up to the tail so the HAM clock never
                        # throttles before the stage-B matmuls
                        emit_outA(kt - 10)
                    # AV lags scores by two k-tiles so the PE never waits on
                    # the exp engines.
                    if kt > 1:
                        emit_av(kt - 2)
                emit_av(14)
                emit_av(15)
                if hp == 3:
                    # last two stage-A units bridge the PE-idle window while
                    # the trailing exps and the pair-3 reciprocal chain
                    # drain, so the clock stays warm into stage B
                    emit_outA(6)
                    emit_outA(7)

                if hp == 3:
                    # stage B consumes pair 3 unnormalized (scaled per head
                    # by the transposed reciprocal after its projection).
                    # Denominators: engine writes must start at an aligned
                    # partition, so stage the four row-HD pieces on
                    # partition 0, then one DMA scatter into sums rows 6:8.
                    # s3 (denominators) on DVE so the reciprocal chain is
                    # not queued behind ACT's trailing exps; outT copies on
                    # ACT (needed later, by the stage-B matmuls)
                    # s3 on ACT (its exp queue drains first, so the
                    # reciprocal chain starts immediately); outT copies on
                    # DVE in parallel
                    s3 = rpool.tile([1, 2, 2, 512], f32, tag="s3", name="s3")
                    for hh in range(2):
                        for qc in range(2):
                            nc.scalar.activation(
                                s3[:, hh, qc, :],
                                av[hh][qc][HD : HD + 1, :],
                                AF.Copy,
                            )
                    for hh in range(2):
                        for qc in range(2):
                            nc.vector.tensor_copy(
                                pair_out[64 * hh : 64 * hh + HD,
                                         qc * 512 : (qc + 1) * 512],
                                av[hh][qc][0:HD, :],
                            )
                    # NOTE: partition-expanding rearrange DMAs are broken on
                    # HW (sim-only); use one shape-preserving DMA per row.
                    for hh in range(2):
                        nc.gpsimd.dma_start(
                            sums_sb[6 + hh : 7 + hh, :],
                            s3[:, hh, :, :].rearrange("p q s -> p (q s)"),
                        )
                else:
                    # drain PSUM accumulators (65 rows incl. the denominator
                    # row); hh=0 on ACT, hh=1 on DVE; denominator row to
                    # sums_sb via the gpsimd DMA queue (SBUF->SBUF).
                    avsbs = []
                    for hh in range(2):
                        avsb = rpool.tile([HD + 1, SQ], f32, tag="avsb",
                                          name=f"avsb{hp}_{hh}")
                        avsbs.append(avsb)
                        for qc in range(2):
                            if hh == 0:
                                nc.scalar.activation(
                                    avsb[:, qc * 512 : (qc + 1) * 512],
                                    av[hh][qc][:], AF.Copy,
                                )
                            else:
                                nc.vector.tensor_copy(
                                    avsb[:, qc * 512 : (qc + 1) * 512],
                                    av[hh][qc][:],
                                )
                        nc.gpsimd.dma_start(
                            sums_sb[2 * hp + hh : 2 * hp + hh + 1, :],
                            avsb[HD : HD + 1, :],
                        )
                    pending_norm[hp] = (pair_out, avsbs)

            # ---- output projection stage B ------------------------------
            # y[q, o] = y_acc[q, o] + sum_{pair3} outT3[e, q] WoT[e, o]/d3
            rcp3 = emit_recip(3)
            for stq in range(8):
                ysb = rpool.tile([128, 512], bf16, tag="ysb", name=f"ysb{stq}")
                for hh in range(2):
                    psy = psB.tile([128, 512], f32, tag="psB",
                                   name=f"psyB{stq}_{hh}")
                    nc.tensor.matmul(
                        psy[:],
                        lhsT=outT[3][64 * hh : 64 * hh + 64,
                                     stq * 128 : (stq + 1) * 128],
                        rhs=wo_sb[64 * hh : 64 * hh + 64, 3, :],
                        start=True,
                        stop=True,
                    )
                    nc.vector.scalar_tensor_tensor(
                        ysb[:],
                        psy[:],
                        rcp3[:, stq, 6 + hh : 7 + hh],
                        y_acc[:, stq, :] if hh == 0 else ysb[:],
                        OP.mult,
                        OP.add,
                    )
                nc.sync.dma_start(y[stq * 128 : (stq + 1) * 128, :], ysb[:])

    nc.compile()
    return nc


def _get_nc():
    if "nc" not in _cache:
        _cache["nc"] = _build()
    return _cache["nc"]


def _host_prep(query, key, value, Wq, bq, Wk, bk, Wv, bv, Wo, bo):
    """Shard + transpose + cast inputs for the 8 cores."""
    bf = ml_dtypes.bfloat16
    f8 = ml_dtypes.float8_e4m3
    wqT = np.ascontiguousarray(Wq.T * 32.0).astype(f8)
    wkT = np.ascontiguousarray(Wk.T * 32.0).astype(f8)
    wvT = np.ascontiguousarray(Wv.T).astype(bf)
    woT = np.ascontiguousarray(Wo.T).astype(bf)
    bqr = np.ascontiguousarray((bq * 0.125).reshape(4, 128).T).astype(np.float32)
    bkr = np.ascontiguousarray(bk.reshape(4, 128).T).astype(np.float32)
    bop = (bo + Wo @ bv).astype(np.float32).reshape(1, D).astype(bf)
    ident = np.eye(128, dtype=np.float32)

    in_maps = []
    for c in range(N_CORES):
        b, half = divmod(c, 2)
        xqT = np.ascontiguousarray(
            query[b, half * SQ : (half + 1) * SQ, :].T
        ).astype(f8)
        keyT = np.ascontiguousarray(key[b].T).astype(f8)
        valT = np.ascontiguousarray(value[b].T).astype(bf)
        in_maps.append(
            {
                "xqT": xqT, "keyT": keyT, "valT": valT,
                "wqT": wqT, "wkT": wkT, "wvT": wvT, "woT": woT,
                "bqr": bqr, "bkr": bkr, "bop": bop, "ident": ident,
            }
        )
    return in_maps


def _assemble(results):
    out = np.empty((B, S, D), np.float32)
    for c in range(N_CORES):
        b, half = divmod(c, 2)
        out[b, half * SQ : (half + 1) * SQ, :] = results[c]["y"]
    return out


def _run(in_maps, **spmd_kwargs):
    from concourse.bass_utils import run_bass_kernel_spmd

    nc = _get_nc()
    return run_bass_kernel_spmd(nc, in_maps, list(range(N_CORES)), **spmd_kwargs)


def _reference_fallback(query, key, value, mask, Wq, bq, Wk, bk, Wv, bv, Wo, bo):
    """Exact numpy path, used only if the mask is not all-ones."""
    q = (query @ Wq.T + bq).reshape(B, S, H, HD).transpose(0, 2, 1, 3)
    k = (key @ Wk.T + bk).reshape(B, S, H, HD).transpose(0, 2, 1, 3)
    v = (value @ Wv.T + bv).reshape(B, S, H, HD).transpose(0, 2, 1, 3)
    scores = np.einsum("bhqd,bhkd->bhqk", q, k) / np.sqrt(HD).astype(np.float32)
    scores = np.where(mask[:, None, :, :] == 0, -np.inf, scores)
    scores = scores - scores.max(axis=-1, keepdims=True)
    e = np.exp(scores)
    attn = e / e.sum(axis=-1, keepdims=True)
    x = np.einsum("bhqk,bhkd->bhqd", attn, v)
    x = x.transpose(0, 2, 1, 3).reshape(B, S, D)
    return (x @ Wo.T + bo).astype(np.float32)


def kernel(query, key, value, mask, Wq, bq, Wk, bk, Wv, bv, Wo, bo):
    query = np.asarray(query, np.float32)
    key = np.asarray(key, np.float32)
    value = np.asarray(value, np.float32)
    mask_np = np.asarray(mask)
    args = [
        np.asarray(a, np.float32)
        for a in (Wq, bq, Wk, bk, Wv, bv, Wo, bo)
    ]
    if not np.all(mask_np != 0):
        return _reference_fallback(query, key, value, mask_np, *args)
    in_maps = _host_prep(query, key, value, *args)
    res = _run(in_maps, trace=False)
    return _assemble(res.results)
